# revision 94
# baseline (speedup 1.0000x reference)
"""Trainium2 Bass kernel for AnchorGNNPocket (GNN message passing), sparse-edge
formulation.

Data-parallel over batch B=8: one complex per NeuronCore (no collectives). The
cutoff graph is ~12% dense (max 8688 of 65536 edges), so instead of the dense
[N,N] edge MLP the active edges are packed into E_CAP columns and the whole
edge pipeline runs on [128, E] tiles (1.35 ms dense baseline -> ~0.21 ms):

- Host extracts the edge list (i_e, j_e, d2_e) per sample, sorts it into 4
  groups by (i>=128, j>=128) with fixed per-group capacities, and builds
  fp8 one-hot gather matrices Si/Sj (the ha[i_e]+hb[j_e] broadcast becomes PE
  matmuls), d2 hi/lo rows for a K=3 bf16 matmul (fp32-class accuracy for the
  wc*d2 term), and the fp8 half-width scatter matrix A (padding edges have
  all-zero rows -> contribute 0).
- Per 512-edge tile, a 3-matmul PSUM group (group-split picks the single
  128-row half of ha_rows/hb_rows needed) builds
  pre[h, e] = ha[:,i_e] + hb[:,j_e] + wc*d2_e; relu(+be1) is one ScalarE
  activation (be1 is per-partition in this layout). Pre-gather is emitted one
  tile ahead of the rest of the pipeline.
- Per 128-edge chunk: the second edge-MLP matmul uses rpre as the STATIONARY
  operand so m1 lands [e-part, h-free] in PSUM; then relu (ScalarE), the
  attention logit via DVE mult-reduce against a replicated Wat (accum_out),
  batched sigmoid, a single broadcast-AP gate multiply (sig column
  broadcast over each chunk via a stride-0 3D view), and a scatter
  matmul (m_g stationary, A chunk moving) accumulating all chunks of an
  i-half into one [128, 256] PSUM tile = aggT. Scatter is emitted one tile
  late so the PE (strict FIFO) never heads-of-line blocks on the gate chain.
- 1/NORM is folded into Wn1b on the host; node MLP + head run in fp32 for
  accuracy (bf16 there costs ~3x the final error), column-split so the next
  layer's ha_rows/hb_rows matmuls start after the first half. S/A matrices
  stream in 6 DMA pieces each, ordered by first use.
"""

import os
import sys

import numpy as np

if not any(os.path.isdir(os.path.join(p, "concourse")) for p in sys.path if p):
    sys.path.insert(0, "/opt/trn_rl_repo")

# ---- problem constants (hardcoded per contest rules) ----
B, NS, NP = 8, 32, 224
N = NS + NP                      # 256 nodes
LIG_NF, POK_NF, JNF, HID, OUT_NF, NLAYERS = 10, 25, 32, 128, 128, 4
CUT2 = 4.5 ** 2
NORM = 100.0

_F32 = np.float32
# edges sorted into 4 groups by (i>=128, j>=128); per-group capacity padded to
# 512 (max over the fixed 8 samples + margin). The group determines which
# 128-row half of ha_rows/hb_rows a tile gathers from, so pre-gather needs
# only 3 matmuls per 512-edge tile (ha + hb + d2).
GROUP_CAPS = (3072, 2048, 2048, 2048)
E_CAP = sum(GROUP_CAPS)          # 9216 = 18 tiles x 512 = 72 chunks x 128
NTILE = E_CAP // 512
NCHUNK = E_CAP // 128
_GOFF = (0, 3072, 5120, 7168, 9216)
_TILE_GROUP = tuple(
    next(g for g in range(4) if _GOFF[g] <= t * 512 < _GOFF[g + 1])
    for t in range(NTILE)
)
_NS_ = 6                         # S-matrix DMA pieces
_TPP = NTILE // _NS_             # tiles per S piece (3)
_NA = 6                          # A-matrix DMA pieces
_CPP = NCHUNK // _NA             # chunks per A piece (12)
_CHALF = _GOFF[2] // 128         # first chunk whose edges target i >= 128


def _np_silu(x):
    return x / (1.0 + np.exp(-x))


def _host_prep(inputs):
    """Embedding h0, pairwise d2, adjacency -> per-sample packed edge data."""
    x = np.concatenate([inputs["mol_x"], inputs["pocket_x"]], axis=1).astype(_F32)
    mask = np.concatenate([inputs["node_mask"], inputs["pocket_mask"]], axis=1).astype(
        _F32
    )
    hm = _np_silu(inputs["mol_h"].astype(_F32) @ inputs["W_mol"] + inputs["b_mol"])
    hp = _np_silu(
        inputs["pocket_h"].astype(_F32) @ inputs["W_pok"] + inputs["b_pok"]
    )
    h0 = (
        np.concatenate([hm, hp], axis=1) @ inputs["W_emb"] + inputs["b_emb"]
    ).astype(_F32)  # [B, N, H]

    diff = x[:, :, None, :] - x[:, None, :, :]
    d2 = np.sum(diff * diff, axis=-1, dtype=_F32)  # [B, N, N]
    idx = np.arange(N)
    lig_pair = (idx[:, None] < NS) & (idx[None, :] < NS)
    adj = np.where(lig_pair, True, d2 <= CUT2)
    adj = adj & (mask[:, :, None] > 0) & (mask[:, None, :] > 0)
    return h0, d2, adj, mask


def _pack_edges(d2_s, adj_s):
    """One sample's graph -> (Sip, Sjp, d2p3, A), group-sorted + padded."""
    import ml_dtypes

    bf = ml_dtypes.bfloat16
    f8 = ml_dtypes.float8_e4m3
    ii0, jj0 = np.nonzero(adj_s)
    g = (ii0 >= HID) * 2 + (jj0 >= HID)
    ii = np.zeros((E_CAP,), dtype=np.int64)
    jj = np.zeros((E_CAP,), dtype=np.int64)
    live = np.zeros((E_CAP,), dtype=bool)
    for gg in range(4):
        sel = g == gg
        ng = int(sel.sum())
        assert ng <= GROUP_CAPS[gg], f"group {gg} over cap: {ng}"
        o = _GOFF[gg]
        ii[o : o + ng] = ii0[sel]
        jj[o : o + ng] = jj0[sel]
        live[o : o + ng] = True
    e = np.arange(E_CAP)[live]

    Sip = np.zeros((HID, E_CAP), dtype=f8)
    Sjp = np.zeros((HID, E_CAP), dtype=f8)
    Sip[ii[e] % HID, e] = 1.0
    Sjp[jj[e] % HID, e] = 1.0

    dvals = np.zeros((E_CAP,), dtype=_F32)
    dvals[e] = d2_s[ii[e], jj[e]]
    hi = dvals.astype(bf)
    lop = (dvals - hi.astype(_F32)).astype(bf)
    d2p3 = np.stack([hi, lop, hi])  # rows pair lhsT [wc_hi, wc_hi, wc_lo]

    # half-width scatter: chunk cc's edges all land in one i-half (group sort)
    A = np.zeros((128, NCHUNK * HID), dtype=f8)
    A[e % 128, (e // 128) * HID + ii[e] % HID] = 1.0
    return Sip, Sjp, d2p3, A


# weight-pack layout
_PLB = 4 * HID          # bf16/layer: wa | wb | We2 | WatB
_PL = 3 * HID + 3       # fp32/layer: Wn1a | Wn1b/NORM | Wn2 | be1, bn1, bn2
_W_COLS = NLAYERS * _PL + HID + 3   # + W_out | W_lin | b_out | b_lin


def _pack_weights(inputs):
    import ml_dtypes

    bf = ml_dtypes.bfloat16
    wpb = np.zeros((HID, NLAYERS * _PLB), dtype=bf)
    wp = np.zeros((HID, _W_COLS), dtype=_F32)
    wc3 = np.zeros((3, NLAYERS * HID), dtype=bf)
    becb = np.zeros((HID, NLAYERS * 512), dtype=_F32)
    bats = np.zeros((NLAYERS,), dtype=_F32)
    be2z = np.zeros((NLAYERS,), dtype=bool)
    We1 = inputs["We1"].astype(_F32)
    for l in range(NLAYERS):
        ob = l * _PLB
        wpb[:, ob : ob + HID] = We1[l, :HID, :]                 # wa
        wpb[:, ob + HID : ob + 2 * HID] = We1[l, HID : 2 * HID, :]  # wb
        wpb[:, ob + 2 * HID : ob + 3 * HID] = inputs["We2"][l]
        wpb[:, ob + 3 * HID : ob + 4 * HID] = np.repeat(
            inputs["Wat"][l].astype(_F32).T, HID, axis=0
        )  # WatB[p, h] = Wat[h]
        o = l * _PL
        wp[:, o : o + HID] = inputs["Wn1"][l][:HID, :]
        wp[:, o + HID : o + 2 * HID] = inputs["Wn1"][l][HID:, :] / NORM
        wp[:, o + 2 * HID : o + 3 * HID] = inputs["Wn2"][l]
        wp[:, o + 3 * HID + 0] = inputs["be1"][l]
        wp[:, o + 3 * HID + 1] = inputs["bn1"][l]
        wp[:, o + 3 * HID + 2] = inputs["bn2"][l]
        wcr = We1[l, 2 * HID, :].astype(_F32)
        whi = wcr.astype(bf)
        wlo = (wcr - whi.astype(_F32)).astype(bf)
        # pairs with d2p3 rows [hi, lo, hi]: whi*hi + whi*lo + wlo*hi
        wc3[0, l * HID : (l + 1) * HID] = whi
        wc3[1, l * HID : (l + 1) * HID] = whi
        wc3[2, l * HID : (l + 1) * HID] = wlo
        becb[:, l * 512 : (l + 1) * 512] = np.tile(inputs["be2"][l], 4)[None, :]
        bats[l] = float(np.asarray(inputs["bat"][l]).reshape(-1)[0])
        be2z[l] = not np.any(np.asarray(inputs["be2"][l]))
    o = NLAYERS * _PL
    wp[:, o : o + HID] = inputs["W_out"].astype(_F32)
    wp[:, o + HID] = inputs["W_lin"][:, 0]
    wp[:, o + HID + 1] = inputs["b_out"]
    wp[0, o + HID + 2] = inputs["b_lin"][0]
    return wpb, wp, wc3, becb, bats, be2z


def _build(nc, tile_mod, bass_mod, n_layers, bats, be2z):
    """Trace the per-core sparse kernel into nc (a Bacc)."""
    mybir = __import__("concourse.mybir", fromlist=["mybir"])
    dt = mybir.dt.float32
    bf = mybir.dt.bfloat16
    AF = mybir.ActivationFunctionType
    ALU = mybir.AluOpType

    hTb_d = nc.dram_tensor("hT0b", [HID, N], bf, kind="ExternalInput")
    f8 = mybir.dt.float8e4
    si_d = [
        nc.dram_tensor(f"Sit{p}", [HID, _TPP * 512], f8, kind="ExternalInput")
        for p in range(_NS_)
    ]
    sj_d = [
        nc.dram_tensor(f"Sjt{p}", [HID, _TPP * 512], f8, kind="ExternalInput")
        for p in range(_NS_)
    ]
    a_d = [
        nc.dram_tensor(f"Ascat{p}", [128, _CPP * HID], f8, kind="ExternalInput")
        for p in range(_NA)
    ]
    d2_d = nc.dram_tensor("d2p3", [3, E_CAP], bf, kind="ExternalInput")
    wb_d = [
        nc.dram_tensor(f"wpackb{l}", [HID, _PLB], bf, kind="ExternalInput")
        for l in range(NLAYERS)
    ]
    wp_d = nc.dram_tensor("wpack", [HID, _W_COLS], dt, kind="ExternalInput")
    wc_d = nc.dram_tensor("wc3", [3, NLAYERS * HID], bf, kind="ExternalInput")
    all_be2_zero = all(bool(z) for z in be2z[:n_layers])
    bec_d = None if all_be2_zero else nc.dram_tensor(
        "becb", [HID, NLAYERS * 512], dt, kind="ExternalInput")
    out_d = nc.dram_tensor("out", [1, NS], dt, kind="ExternalOutput")

    with tile_mod.TileContext(nc) as tc:
        with (
            tc.tile_pool(name="const", bufs=1) as cpool,
            tc.tile_pool(name="layer", bufs=2) as lpool,
            tc.tile_pool(name="work", bufs=4) as wpool,
            tc.tile_pool(name="psA", bufs=3, space="PSUM") as psA,
            tc.tile_pool(name="psB", bufs=3, space="PSUM") as psB,
            tc.tile_pool(name="psC", bufs=1, space="PSUM") as psC,
            tc.tile_pool(name="psD", bufs=1, space="PSUM") as psD,
        ):
            # ---- load constants (ordered by first use; S/A split fine so the
            # first tiles' data lands across many parallel DMA queues) ----
            hTb0 = cpool.tile([HID, N], bf, tag="hT0b")
            wpbs = [
                cpool.tile([HID, _PLB], bf, tag=f"wpackb{l}", name=f"wpackb{l}")
                for l in range(NLAYERS)
            ]
            wc3 = cpool.tile([3, NLAYERS * HID], bf, tag="wc3")
            wp = cpool.tile([HID, _W_COLS], dt, tag="wpack")
            d2p = cpool.tile([3, E_CAP], bf, tag="d2p3")
            Sis = [
                cpool.tile([HID, _TPP * 512], f8, tag=f"Sit{p}", name=f"Sit{p}")
                for p in range(_NS_)
            ]
            Sjs = [
                cpool.tile([HID, _TPP * 512], f8, tag=f"Sjt{p}", name=f"Sjt{p}")
                for p in range(_NS_)
            ]
            Ascs = [
                cpool.tile([128, _CPP * HID], f8, tag=f"Ascat{p}", name=f"Ascat{p}")
                for p in range(_NA)
            ]
            dmas = [(d2p, d2_d), (hTb0, hTb_d), (wpbs[0], wb_d[0]),
                    (wc3, wc_d)]
            na_done = 0
            for p in range(_NS_):
                dmas += [(Sis[p], si_d[p]), (Sjs[p], sj_d[p])]
                while na_done < _NA and na_done * _CPP <= (p + 1) * _TPP * 4:
                    dmas.append((Ascs[na_done], a_d[na_done]))
                    na_done += 1
            for l in range(1, NLAYERS):
                dmas.append((wpbs[l], wb_d[l]))
            dmas.append((wp, wp_d))
            if bec_d is not None:
                becb = cpool.tile([HID, NLAYERS * 512], dt, tag="becb")
                dmas.append((becb, bec_d))
            for t, d in dmas:
                nc.sync.dma_start(t[:], d.ap())

            def si_ap(t):
                return Sis[t // _TPP][:, (t % _TPP) * 512 : (t % _TPP + 1) * 512]

            def sj_ap(t):
                return Sjs[t // _TPP][:, (t % _TPP) * 512 : (t % _TPP + 1) * 512]

            def a_ap(cc):
                return Ascs[cc // _CPP][:, (cc % _CPP) * HID : (cc % _CPP + 1) * HID]

            # fp32 copy of h0 for the residual path
            hT = cpool.tile([HID, N], dt, tag="hTf")
            nc.vector.tensor_copy(hT[:], hTb0[:])

            hT_cur, hTb_cur = hT, hTb0
            for l in range(n_layers):
                wpbl = wpbs[l]
                wab = wpbl[:, 0:HID]
                wbb = wpbl[:, HID : 2 * HID]
                We2b = wpbl[:, 2 * HID : 3 * HID]
                WatB = wpbl[:, 3 * HID : 4 * HID]
                o = l * _PL
                Wn1a = wp[:, o : o + HID]
                Wn1b = wp[:, o + HID : o + 2 * HID]
                Wn2 = wp[:, o + 2 * HID : o + 3 * HID]
                be1 = wp[:, o + 3 * HID : o + 3 * HID + 1]
                bn1 = wp[:, o + 3 * HID + 1 : o + 3 * HID + 2]
                bn2 = wp[:, o + 3 * HID + 2 : o + 3 * HID + 3]
                wc3l = wc3[:, l * HID : (l + 1) * HID]
                be2_zero = bool(be2z[l])
                be2b = None if bec_d is None else becb[:, l * 512 : (l + 1) * 512]

                # ---- ha_rows/hb_rows: [i-part, h-free], bf16 ----
                ps_h0 = psA.tile([HID, 2 * HID], dt, tag="pre")
                nc.tensor.matmul(ps_h0[:, 0:HID], hTb_cur[:, 0:HID], wab,
                                 start=True, stop=True)
                nc.tensor.matmul(ps_h0[:, HID : 2 * HID], hTb_cur[:, 0:HID], wbb,
                                 start=True, stop=True)
                ps_h1 = psA.tile([HID, 2 * HID], dt, tag="pre")
                nc.tensor.matmul(ps_h1[:, 0:HID], hTb_cur[:, HID:N], wab,
                                 start=True, stop=True)
                nc.tensor.matmul(ps_h1[:, HID : 2 * HID], hTb_cur[:, HID:N], wbb,
                                 start=True, stop=True)
                har0 = lpool.tile([HID, 2 * HID], bf, tag="har0")
                har1 = lpool.tile([HID, 2 * HID], bf, tag="har1")
                nc.vector.tensor_copy(har0[:], ps_h0[:])
                nc.vector.tensor_copy(har1[:], ps_h1[:])

                attc = lpool.tile([HID, NCHUNK], dt, tag="attc")
                sigc = lpool.tile([HID, NCHUNK], dt, tag="sigc")
                ps_agg = psC.tile([HID, N], dt, tag="agg")
                # node-MLP h-term can run any time this layer
                ps_n1 = psD.tile([HID, N], dt, tag="n1")
                aggb = lpool.tile([HID, N], dt, tag="aggb")

                # ---- edge tiles (pre-gather software-pipelined one tile
                # ahead so the PE never stalls on the rpre relu) ----
                def emit_pre(t):
                    g = _TILE_GROUP[t]
                    hari = (har0, har1)[g >> 1]
                    harj = (har0, har1)[g & 1]
                    ps_pre = psA.tile([HID, 512], dt, tag="pre", name=f"pre{t}")
                    nc.tensor.matmul(ps_pre[:], hari[:, 0:HID], si_ap(t),
                                     start=True, stop=False)
                    nc.tensor.matmul(ps_pre[:], harj[:, HID : 2 * HID], sj_ap(t),
                                     start=False, stop=False)
                    nc.tensor.matmul(ps_pre[:], wc3l, d2p[:, t * 512 : (t + 1) * 512],
                                     start=False, stop=True)
                    rpre = wpool.tile([HID, 512], bf, tag="rpre", bufs=6)
                    nc.scalar.activation(rpre[:], ps_pre[:], AF.Relu, bias=be1)
                    return rpre

                def emit_scatter(t, mg):
                    ihalf = _TILE_GROUP[t] >> 1
                    hsl = slice(ihalf * HID, (ihalf + 1) * HID)
                    for k in range(4):
                        ck = slice(k * HID, (k + 1) * HID)
                        cc = t * 4 + k
                        nc.tensor.matmul(
                            ps_agg[:, hsl], mg[:, ck], a_ap(cc),
                            start=(cc == 0 or cc == _CHALF),
                            stop=(cc == _CHALF - 1 or cc == NCHUNK - 1),
                        )

                rpre_next = emit_pre(0)
                sc_prev = None
                for t in range(NTILE):
                    g = _TILE_GROUP[t]
                    rpre = rpre_next
                    if t + 1 < NTILE:
                        rpre_next = emit_pre(t + 1)

                    ps_m1 = psB.tile([128, 512], dt, tag="m1", name=f"m1_{t}")
                    for k in range(4):
                        ck = slice(k * HID, (k + 1) * HID)
                        nc.tensor.matmul(ps_m1[:, ck], rpre[:, ck], We2b,
                                         start=True, stop=True)
                    m = wpool.tile([128, 512], bf, tag="m", bufs=6)
                    if be2_zero:
                        if t % 2 == 0:
                            nc.scalar.activation(m[:], ps_m1[:], AF.Relu, bias=0.0)
                        else:
                            nc.vector.tensor_scalar_max(m[:], ps_m1[:], 0.0)
                    else:
                        m1s = wpool.tile([128, 512], dt, tag="m1s", bufs=3)
                        nc.vector.tensor_tensor(m1s[:], ps_m1[:], be2b, ALU.add)
                        nc.scalar.activation(m[:], m1s[:], AF.Relu, bias=0.0)

                    # att[e] = sum_h m[e,h] * Wat[h] (DVE accum along free dim)
                    scr = wpool.tile([128, 512], bf, tag="scr", bufs=4)
                    for k in range(4):
                        ck = slice(k * HID, (k + 1) * HID)
                        cc = t * 4 + k
                        nc.vector.scalar_tensor_tensor(
                            out=scr[:, ck], in0=m[:, ck], scalar=1.0,
                            in1=WatB, op0=ALU.mult, op1=ALU.mult,
                            accum_out=attc[:, cc : cc + 1],
                        )
                    nc.scalar.activation(
                        sigc[:, t * 4 : (t + 1) * 4], attc[:, t * 4 : (t + 1) * 4],
                        AF.Sigmoid, bias=float(bats[l]),
                    )
                    mg = wpool.tile([128, 512], bf, tag="mg", bufs=6)
                    sl4 = slice(t * 4, (t + 1) * 4)
                    nc.vector.tensor_tensor(
                        mg[:].rearrange("p (k h) -> p k h", k=4),
                        m[:].rearrange("p (k h) -> p k h", k=4),
                        sigc[:, sl4].unsqueeze(-1).broadcast_to([HID, 4, HID]),
                        ALU.mult,
                    )
                    # scatter deferred one iteration so its inputs are ready
                    # well before the PE (strict FIFO) reaches it
                    if sc_prev is not None:
                        emit_scatter(*sc_prev)
                    sc_prev = (t, mg)
                emit_scatter(*sc_prev)
                # ---- node MLP:  h += relu([h, agg] @ Wn1 + bn1) @ Wn2 + bn2 ----
                # column-separable: finish i<128 first so the next layer's
                # har matmuls (which consume hTb by column half) start sooner
                nc.vector.tensor_copy(aggb[:], ps_agg[:])
                nc.tensor.matmul(ps_n1[:], Wn1a, hT_cur[:], start=True, stop=False)
                nc.tensor.matmul(ps_n1[:], Wn1b, aggb[:], start=False, stop=True)
                t1 = wpool.tile([HID, N], dt, tag="m1s")
                ps_n2 = psB.tile([HID, N], dt, tag="m1")
                hT_new = lpool.tile([HID, N], dt, tag="hT")
                hTb_new = lpool.tile([HID, N], bf, tag="hTb")
                for hs in (slice(0, HID), slice(HID, N)):
                    nc.scalar.activation(t1[:, hs], ps_n1[:, hs], AF.Relu, bias=bn1)
                    nc.tensor.matmul(ps_n2[:, hs], Wn2, t1[:, hs],
                                     start=True, stop=True)
                    nc.vector.scalar_tensor_tensor(
                        out=hT_new[:, hs], in0=ps_n2[:, hs], scalar=bn2,
                        in1=hT_cur[:, hs], op0=ALU.add, op1=ALU.add,
                    )
                    nc.vector.tensor_copy(hTb_new[:, hs], hT_new[:, hs])
                hT_cur, hTb_cur = hT_new, hTb_new

            # ---- output head ----
            o = NLAYERS * _PL
            W_out = wp[:, o : o + HID]
            W_lin = wp[:, o + HID : o + HID + 1]
            b_out = wp[:, o + HID + 1 : o + HID + 2]
            b_lin = wp[0:1, o + HID + 2 : o + HID + 3]
            ps_o = psA.tile([HID, NS], dt, tag="pre")
            nc.tensor.matmul(ps_o[:], W_out, hT_cur[:, 0:NS], start=True, stop=True)
            ho = wpool.tile([HID, NS], dt, tag="m1s")
            nc.scalar.activation(ho[:], ps_o[:], AF.Relu, bias=b_out)
            ps_y = psB.tile([1, NS], dt, tag="m1")
            nc.tensor.matmul(ps_y[:], W_lin, ho[:], start=True, stop=True)
            y = wpool.tile([1, NS], dt, tag="scr", bufs=4)
            nc.scalar.activation(y[:], ps_y[:], AF.Identity, bias=b_lin)
            nc.sync.dma_start(out_d.ap(), y[:])


def _make_in_maps(inputs, n_layers):
    import ml_dtypes

    bf = ml_dtypes.bfloat16
    h0, d2, adj, mask = _host_prep(inputs)
    wpb, wp, wc3, becb, bats, be2z = _pack_weights(inputs)
    in_maps = []
    for b in range(B):
        Sip, Sjp, d2p3, A = _pack_edges(d2[b], adj[b])
        hTb = np.ascontiguousarray(h0[b].T)
        im = {
            "hT0b": hTb.astype(bf),
            "d2p3": d2p3,
            "wpack": wp, "wc3": wc3,
        }
        for l in range(NLAYERS):
            im[f"wpackb{l}"] = np.ascontiguousarray(
                wpb[:, l * _PLB : (l + 1) * _PLB])
        if not all(bool(z) for z in be2z):
            im["becb"] = becb
        for p in range(_NS_):
            im[f"Sit{p}"] = np.ascontiguousarray(
                Sip[:, p * _TPP * 512 : (p + 1) * _TPP * 512])
            im[f"Sjt{p}"] = np.ascontiguousarray(
                Sjp[:, p * _TPP * 512 : (p + 1) * _TPP * 512])
        for p in range(_NA):
            im[f"Ascat{p}"] = np.ascontiguousarray(
                A[:, p * _CPP * HID : (p + 1) * _CPP * HID])
        in_maps.append(im)
    return in_maps, mask, bats, be2z


def _install_ntff_hook():
    """Recreate the antenv.axon_hooks module the boot expected, register the
    ctypes NTFF hook from trn_agent_boot, so run_bass_kernel_spmd(trace=True)
    can capture hardware profiles under axon."""
    import types

    if "antenv.axon_hooks" not in sys.modules:
        mod = types.ModuleType("antenv.axon_hooks")
        holder = [None]
        mod.set_axon_ntff_profile_hook = lambda h: holder.__setitem__(0, h)
        mod.get_axon_ntff_profile_hook = lambda: holder[0]
        sys.modules["antenv.axon_hooks"] = mod
        import antenv

        antenv.axon_hooks = mod
    m = sys.modules["antenv.axon_hooks"]
    if m.get_axon_ntff_profile_hook() is None:
        sys.path.insert(0, "/root/.axon_site")
        from trn_agent_boot.trn_boot import _ntff_profile_via_ctypes

        m.set_axon_ntff_profile_hook(
            _ntff_profile_via_ctypes("/opt/axon/libaxon_pjrt.so")
        )


_CACHE = {}


def _get_nc(n_layers, bats, be2z):
    key = (n_layers, tuple(np.round(bats, 8)), tuple(be2z))
    if key not in _CACHE:
        import concourse.bass as bass
        import concourse.tile as tile
        from concourse import bacc

        nc = bacc.Bacc(
            "TRN2", target_bir_lowering=False, debug=False, num_devices=B
        )
        _build(nc, tile, bass, n_layers, bats, be2z)
        nc.compile()
        _CACHE[key] = nc
    return _CACHE[key]


def kernel(**inputs):
    inputs = {k: np.asarray(v) for k, v in inputs.items()}
    n_layers = int(os.environ.get("GNN_LAYERS", NLAYERS))
    in_maps, mask, bats, be2z = _make_in_maps(inputs, n_layers)
    nc = _get_nc(n_layers, bats, be2z)

    if os.environ.get("GNN_SIM"):
        from concourse.bass_interp import CoreSim

        outs = []
        for b in range(int(os.environ.get("GNN_SIM_CORES", 1))):
            sim = CoreSim(nc, trace=False)
            for k, v in in_maps[b].items():
                sim.tensor(k)[:] = v
            sim.simulate()
            outs.append(np.array(sim.tensor("out")).reshape(NS, 1))
        while len(outs) < B:
            outs.append(np.zeros((NS, 1), _F32))
        out = np.stack(outs)
    else:
        from concourse.bass_utils import run_bass_kernel_spmd

        if os.environ.get("GNN_TRACE"):
            _install_ntff_hook()
            tmpdir = os.environ.get("GNN_TRACE_DIR") or None
            try:
                res = run_bass_kernel_spmd(
                    nc, in_maps, core_ids=list(range(B)), trace=True, tmpdir=tmpdir
                )
                kernel.last_exec_time_ns = res.exec_time_ns
            except Exception as e:
                print(f"[gnn] traced run failed ({e!r}); retrying untraced")
                res = run_bass_kernel_spmd(nc, in_maps, core_ids=list(range(B)))
        else:
            res = run_bass_kernel_spmd(nc, in_maps, core_ids=list(range(B)))
        kernel.last_results = res
        out = np.stack([r["out"].reshape(NS, 1) for r in res.results])

    return (out * inputs["node_mask"][:, :, None]).astype(_F32)


# revision 95
# speedup vs baseline: 1.1522x; 1.1522x over previous
"""Trainium2 Bass kernel for AnchorGNNPocket (GNN message passing), sparse-edge
formulation.

Data-parallel over batch B=8: one complex per NeuronCore (no collectives). The
cutoff graph is ~12% dense (max 8688 of 65536 edges), so instead of the dense
[N,N] edge MLP the active edges are packed into E_CAP columns and the whole
edge pipeline runs on [128, E] tiles (1.35 ms dense baseline -> ~0.21 ms):

- Host extracts the edge list (i_e, j_e, d2_e) per sample, sorts it into 4
  groups by (i>=128, j>=128) with fixed per-group capacities, and builds
  fp8 one-hot gather matrices Si/Sj (the ha[i_e]+hb[j_e] broadcast becomes PE
  matmuls), d2 hi/lo rows for a K=3 bf16 matmul (fp32-class accuracy for the
  wc*d2 term), and the fp8 half-width scatter matrix A (padding edges have
  all-zero rows -> contribute 0).
- Per 512-edge tile, a 3-matmul PSUM group (group-split picks the single
  128-row half of ha_rows/hb_rows needed) builds
  pre[h, e] = ha[:,i_e] + hb[:,j_e] + wc*d2_e; relu(+be1) is one ScalarE
  activation (be1 is per-partition in this layout). Pre-gather is emitted one
  tile ahead of the rest of the pipeline.
- Per 128-edge chunk: the second edge-MLP matmul uses rpre as the STATIONARY
  operand so m1 lands [e-part, h-free] in PSUM; then relu (ScalarE), the
  attention logit via DVE mult-reduce against a replicated Wat (accum_out),
  batched sigmoid, a single broadcast-AP gate multiply (sig column
  broadcast over each chunk via a stride-0 3D view), and a scatter
  matmul (m_g stationary, A chunk moving) accumulating all chunks of an
  i-half into one [128, 256] PSUM tile = aggT. Scatter is emitted one tile
  late so the PE (strict FIFO) never heads-of-line blocks on the gate chain.
- 1/NORM is folded into Wn1b on the host; node MLP + head run in fp32 for
  accuracy (bf16 there costs ~3x the final error), column-split so the next
  layer's ha_rows/hb_rows matmuls start after the first half. S/A matrices
  stream in 6 DMA pieces each, ordered by first use.
"""

import os
import sys

import numpy as np

if not any(os.path.isdir(os.path.join(p, "concourse")) for p in sys.path if p):
    sys.path.insert(0, "/opt/trn_rl_repo")

# ---- problem constants (hardcoded per contest rules) ----
B, NS, NP = 8, 32, 224
N = NS + NP                      # 256 nodes
LIG_NF, POK_NF, JNF, HID, OUT_NF, NLAYERS = 10, 25, 32, 128, 128, 4
CUT2 = 4.5 ** 2
NORM = 100.0

_F32 = np.float32
# edges sorted into 4 groups by (i>=128, j>=128); per-group capacity padded to
# 512 (max over the fixed 8 samples + margin). The group determines which
# 128-row half of ha_rows/hb_rows a tile gathers from, so pre-gather needs
# only 3 matmuls per 512-edge tile (ha + hb + d2).
GROUP_CAPS = (3072, 2048, 2048, 2048)
E_CAP = sum(GROUP_CAPS)          # 9216 = 18 tiles x 512 = 72 chunks x 128
NTILE = E_CAP // 512
NCHUNK = E_CAP // 128
_GOFF = (0, 3072, 5120, 7168, 9216)
_TILE_GROUP = tuple(
    next(g for g in range(4) if _GOFF[g] <= t * 512 < _GOFF[g + 1])
    for t in range(NTILE)
)
_SB = (0, 1, 3, 6, 10, 14, 18)   # S-piece tile boundaries (small first)
_NS_ = len(_SB) - 1
_AB = (0, 4, 12, 24, 40, 56, 72)  # A-piece chunk boundaries
_NA = len(_AB) - 1
_CHALF = _GOFF[2] // 128         # first chunk whose edges target i >= 128


def _np_silu(x):
    return x / (1.0 + np.exp(-x))


def _host_prep(inputs):
    """Embedding h0, pairwise d2, adjacency -> per-sample packed edge data."""
    x = np.concatenate([inputs["mol_x"], inputs["pocket_x"]], axis=1).astype(_F32)
    mask = np.concatenate([inputs["node_mask"], inputs["pocket_mask"]], axis=1).astype(
        _F32
    )
    hm = _np_silu(inputs["mol_h"].astype(_F32) @ inputs["W_mol"] + inputs["b_mol"])
    hp = _np_silu(
        inputs["pocket_h"].astype(_F32) @ inputs["W_pok"] + inputs["b_pok"]
    )
    h0 = (
        np.concatenate([hm, hp], axis=1) @ inputs["W_emb"] + inputs["b_emb"]
    ).astype(_F32)  # [B, N, H]

    diff = x[:, :, None, :] - x[:, None, :, :]
    d2 = np.sum(diff * diff, axis=-1, dtype=_F32)  # [B, N, N]
    idx = np.arange(N)
    lig_pair = (idx[:, None] < NS) & (idx[None, :] < NS)
    adj = np.where(lig_pair, True, d2 <= CUT2)
    adj = adj & (mask[:, :, None] > 0) & (mask[:, None, :] > 0)
    return h0, d2, adj, mask


def _pack_edges(d2_s, adj_s):
    """One sample's graph -> (Sip, Sjp, d2p3, A), group-sorted + padded."""
    import ml_dtypes

    bf = ml_dtypes.bfloat16
    f8 = ml_dtypes.float8_e4m3
    ii0, jj0 = np.nonzero(adj_s)
    g = (ii0 >= HID) * 2 + (jj0 >= HID)
    ii = np.zeros((E_CAP,), dtype=np.int64)
    jj = np.zeros((E_CAP,), dtype=np.int64)
    live = np.zeros((E_CAP,), dtype=bool)
    for gg in range(4):
        sel = g == gg
        ng = int(sel.sum())
        assert ng <= GROUP_CAPS[gg], f"group {gg} over cap: {ng}"
        o = _GOFF[gg]
        ii[o : o + ng] = ii0[sel]
        jj[o : o + ng] = jj0[sel]
        live[o : o + ng] = True
    e = np.arange(E_CAP)[live]

    Sip = np.zeros((HID, E_CAP), dtype=f8)
    Sjp = np.zeros((HID, E_CAP), dtype=f8)
    Sip[ii[e] % HID, e] = 1.0
    Sjp[jj[e] % HID, e] = 1.0

    dvals = np.zeros((E_CAP,), dtype=_F32)
    dvals[e] = d2_s[ii[e], jj[e]]
    hi = dvals.astype(bf)
    lop = (dvals - hi.astype(_F32)).astype(bf)
    d2p3 = np.stack([hi, lop, hi])  # rows pair lhsT [wc_hi, wc_hi, wc_lo]

    # half-width scatter: chunk cc's edges all land in one i-half (group sort)
    A = np.zeros((128, NCHUNK * HID), dtype=f8)
    A[e % 128, (e // 128) * HID + ii[e] % HID] = 1.0
    return Sip, Sjp, d2p3, A


# weight-pack layout
_PLB = 4 * HID          # bf16/layer: wa | wb | We2 | WatB
_PL = 3 * HID + 3       # fp32/layer: Wn1a | Wn1b/NORM | Wn2 | be1, bn1, bn2
_W_COLS = NLAYERS * _PL + HID + 3   # + W_out | W_lin | b_out | b_lin


def _pack_weights(inputs):
    import ml_dtypes

    bf = ml_dtypes.bfloat16
    wpb = np.zeros((HID, NLAYERS * _PLB), dtype=bf)
    wp = np.zeros((HID, _W_COLS), dtype=_F32)
    wc3 = np.zeros((3, NLAYERS * HID), dtype=bf)
    becb = np.zeros((HID, NLAYERS * 512), dtype=_F32)
    bats = np.zeros((NLAYERS,), dtype=_F32)
    be2z = np.zeros((NLAYERS,), dtype=bool)
    We1 = inputs["We1"].astype(_F32)
    for l in range(NLAYERS):
        ob = l * _PLB
        wpb[:, ob : ob + HID] = We1[l, :HID, :]                 # wa
        wpb[:, ob + HID : ob + 2 * HID] = We1[l, HID : 2 * HID, :]  # wb
        wpb[:, ob + 2 * HID : ob + 3 * HID] = inputs["We2"][l]
        wpb[:, ob + 3 * HID : ob + 4 * HID] = np.repeat(
            inputs["Wat"][l].astype(_F32).T, HID, axis=0
        )  # WatB[p, h] = Wat[h]
        o = l * _PL
        wp[:, o : o + HID] = inputs["Wn1"][l][:HID, :]
        wp[:, o + HID : o + 2 * HID] = inputs["Wn1"][l][HID:, :] / NORM
        wp[:, o + 2 * HID : o + 3 * HID] = inputs["Wn2"][l]
        wp[:, o + 3 * HID + 0] = inputs["be1"][l]
        wp[:, o + 3 * HID + 1] = inputs["bn1"][l]
        wp[:, o + 3 * HID + 2] = inputs["bn2"][l]
        wcr = We1[l, 2 * HID, :].astype(_F32)
        whi = wcr.astype(bf)
        wlo = (wcr - whi.astype(_F32)).astype(bf)
        # pairs with d2p3 rows [hi, lo, hi]: whi*hi + whi*lo + wlo*hi
        wc3[0, l * HID : (l + 1) * HID] = whi
        wc3[1, l * HID : (l + 1) * HID] = whi
        wc3[2, l * HID : (l + 1) * HID] = wlo
        becb[:, l * 512 : (l + 1) * 512] = np.tile(inputs["be2"][l], 4)[None, :]
        bats[l] = float(np.asarray(inputs["bat"][l]).reshape(-1)[0])
        be2z[l] = not np.any(np.asarray(inputs["be2"][l]))
    o = NLAYERS * _PL
    wp[:, o : o + HID] = inputs["W_out"].astype(_F32)
    wp[:, o + HID] = inputs["W_lin"][:, 0]
    wp[:, o + HID + 1] = inputs["b_out"]
    wp[0, o + HID + 2] = inputs["b_lin"][0]
    return wpb, wp, wc3, becb, bats, be2z


def _build(nc, tile_mod, bass_mod, n_layers, bats, be2z):
    """Trace the per-core sparse kernel into nc (a Bacc)."""
    mybir = __import__("concourse.mybir", fromlist=["mybir"])
    dt = mybir.dt.float32
    bf = mybir.dt.bfloat16
    AF = mybir.ActivationFunctionType
    ALU = mybir.AluOpType

    hTb_d = nc.dram_tensor("hT0b", [HID, N], bf, kind="ExternalInput")
    f8 = mybir.dt.float8e4
    si_d = [
        nc.dram_tensor(f"Sit{p}", [HID, (_SB[p + 1] - _SB[p]) * 512], f8,
                       kind="ExternalInput")
        for p in range(_NS_)
    ]
    sj_d = [
        nc.dram_tensor(f"Sjt{p}", [HID, (_SB[p + 1] - _SB[p]) * 512], f8,
                       kind="ExternalInput")
        for p in range(_NS_)
    ]
    a_d = [
        nc.dram_tensor(f"Ascat{p}", [128, (_AB[p + 1] - _AB[p]) * HID], f8,
                       kind="ExternalInput")
        for p in range(_NA)
    ]
    d2_d = nc.dram_tensor("d2p3", [3, E_CAP], bf, kind="ExternalInput")
    wb_d = [
        nc.dram_tensor(f"wpackb{l}", [HID, _PLB], bf, kind="ExternalInput")
        for l in range(NLAYERS)
    ]
    wp_d = nc.dram_tensor("wpack", [HID, _W_COLS], dt, kind="ExternalInput")
    wc_d = nc.dram_tensor("wc3", [3, NLAYERS * HID], bf, kind="ExternalInput")
    all_be2_zero = all(bool(z) for z in be2z[:n_layers])
    bec_d = None if all_be2_zero else nc.dram_tensor(
        "becb", [HID, NLAYERS * 512], dt, kind="ExternalInput")
    out_d = nc.dram_tensor("out", [1, NS], dt, kind="ExternalOutput")

    with tile_mod.TileContext(nc) as tc:
        with (
            tc.tile_pool(name="const", bufs=1) as cpool,
            tc.tile_pool(name="layer", bufs=2) as lpool,
            tc.tile_pool(name="work", bufs=4) as wpool,
            tc.tile_pool(name="psA", bufs=3, space="PSUM") as psA,
            tc.tile_pool(name="psB", bufs=3, space="PSUM") as psB,
            tc.tile_pool(name="psC", bufs=1, space="PSUM") as psC,
            tc.tile_pool(name="psD", bufs=1, space="PSUM") as psD,
        ):
            # ---- load constants (ordered by first use; S/A split fine so the
            # first tiles' data lands across many parallel DMA queues) ----
            hTb0 = cpool.tile([HID, N], bf, tag="hT0b")
            wpbs = [
                cpool.tile([HID, _PLB], bf, tag=f"wpackb{l}", name=f"wpackb{l}")
                for l in range(NLAYERS)
            ]
            wc3 = cpool.tile([3, NLAYERS * HID], bf, tag="wc3")
            wp = cpool.tile([HID, _W_COLS], dt, tag="wpack")
            d2p = cpool.tile([3, E_CAP], bf, tag="d2p3")
            Sis = [
                cpool.tile([HID, (_SB[p + 1] - _SB[p]) * 512], f8,
                           tag=f"Sit{p}", name=f"Sit{p}")
                for p in range(_NS_)
            ]
            Sjs = [
                cpool.tile([HID, (_SB[p + 1] - _SB[p]) * 512], f8,
                           tag=f"Sjt{p}", name=f"Sjt{p}")
                for p in range(_NS_)
            ]
            Ascs = [
                cpool.tile([128, (_AB[p + 1] - _AB[p]) * HID], f8,
                           tag=f"Ascat{p}", name=f"Ascat{p}")
                for p in range(_NA)
            ]
            dmas = [(d2p, d2_d), (hTb0, hTb_d), (wpbs[0], wb_d[0]),
                    (wc3, wc_d)]
            na_done = 0
            for p in range(_NS_):
                dmas += [(Sis[p], si_d[p]), (Sjs[p], sj_d[p])]
                while na_done < _NA and _AB[na_done] <= _SB[p + 1] * 4:
                    dmas.append((Ascs[na_done], a_d[na_done]))
                    na_done += 1
            for l in range(1, NLAYERS):
                dmas.append((wpbs[l], wb_d[l]))
            dmas.append((wp, wp_d))
            if bec_d is not None:
                becb = cpool.tile([HID, NLAYERS * 512], dt, tag="becb")
                dmas.append((becb, bec_d))
            for t, d in dmas:
                nc.sync.dma_start(t[:], d.ap())

            def _piece(bounds, x):
                p = next(q for q in range(len(bounds) - 1)
                         if bounds[q] <= x < bounds[q + 1])
                return p, x - bounds[p]

            def si_ap(t):
                p, o = _piece(_SB, t)
                return Sis[p][:, o * 512 : (o + 1) * 512]

            def sj_ap(t):
                p, o = _piece(_SB, t)
                return Sjs[p][:, o * 512 : (o + 1) * 512]

            def a_ap(cc):
                p, o = _piece(_AB, cc)
                return Ascs[p][:, o * HID : (o + 1) * HID]

            # fp32 copy of h0 for the residual path
            hT = cpool.tile([HID, N], dt, tag="hTf")
            nc.vector.tensor_copy(hT[:], hTb0[:])

            hT_cur, hTb_cur = hT, hTb0
            for l in range(n_layers):
                wpbl = wpbs[l]
                wab = wpbl[:, 0:HID]
                wbb = wpbl[:, HID : 2 * HID]
                We2b = wpbl[:, 2 * HID : 3 * HID]
                WatB = wpbl[:, 3 * HID : 4 * HID]
                o = l * _PL
                Wn1a = wp[:, o : o + HID]
                Wn1b = wp[:, o + HID : o + 2 * HID]
                Wn2 = wp[:, o + 2 * HID : o + 3 * HID]
                be1 = wp[:, o + 3 * HID : o + 3 * HID + 1]
                bn1 = wp[:, o + 3 * HID + 1 : o + 3 * HID + 2]
                bn2 = wp[:, o + 3 * HID + 2 : o + 3 * HID + 3]
                wc3l = wc3[:, l * HID : (l + 1) * HID]
                be2_zero = bool(be2z[l])
                be2b = None if bec_d is None else becb[:, l * 512 : (l + 1) * 512]

                # ---- ha_rows/hb_rows: [i-part, h-free], bf16 ----
                ps_h0 = psA.tile([HID, 2 * HID], dt, tag="pre")
                nc.tensor.matmul(ps_h0[:, 0:HID], hTb_cur[:, 0:HID], wab,
                                 start=True, stop=True)
                nc.tensor.matmul(ps_h0[:, HID : 2 * HID], hTb_cur[:, 0:HID], wbb,
                                 start=True, stop=True)
                ps_h1 = psA.tile([HID, 2 * HID], dt, tag="pre")
                nc.tensor.matmul(ps_h1[:, 0:HID], hTb_cur[:, HID:N], wab,
                                 start=True, stop=True)
                nc.tensor.matmul(ps_h1[:, HID : 2 * HID], hTb_cur[:, HID:N], wbb,
                                 start=True, stop=True)
                har0 = lpool.tile([HID, 2 * HID], bf, tag="har0")
                har1 = lpool.tile([HID, 2 * HID], bf, tag="har1")
                nc.vector.tensor_copy(har0[:], ps_h0[:])
                nc.vector.tensor_copy(har1[:], ps_h1[:])

                attc = lpool.tile([HID, NCHUNK], dt, tag="attc")
                sigc = lpool.tile([HID, NCHUNK], dt, tag="sigc")
                ps_agg = psC.tile([HID, N], dt, tag="agg")
                # node-MLP h-term can run any time this layer
                ps_n1 = psD.tile([HID, N], dt, tag="n1")
                aggb = lpool.tile([HID, N], dt, tag="aggb")

                # ---- edge tiles (pre-gather software-pipelined one tile
                # ahead so the PE never stalls on the rpre relu) ----
                def emit_pre(t):
                    g = _TILE_GROUP[t]
                    hari = (har0, har1)[g >> 1]
                    harj = (har0, har1)[g & 1]
                    ps_pre = psA.tile([HID, 512], dt, tag="pre", name=f"pre{t}")
                    nc.tensor.matmul(ps_pre[:], hari[:, 0:HID], si_ap(t),
                                     start=True, stop=False)
                    nc.tensor.matmul(ps_pre[:], harj[:, HID : 2 * HID], sj_ap(t),
                                     start=False, stop=False)
                    nc.tensor.matmul(ps_pre[:], wc3l, d2p[:, t * 512 : (t + 1) * 512],
                                     start=False, stop=True)
                    rpre = wpool.tile([HID, 512], bf, tag="rpre", bufs=6)
                    nc.scalar.activation(rpre[:], ps_pre[:], AF.Relu, bias=be1)
                    return rpre

                def emit_scatter(t, mg):
                    ihalf = _TILE_GROUP[t] >> 1
                    hsl = slice(ihalf * HID, (ihalf + 1) * HID)
                    for k in range(4):
                        ck = slice(k * HID, (k + 1) * HID)
                        cc = t * 4 + k
                        nc.tensor.matmul(
                            ps_agg[:, hsl], mg[:, ck], a_ap(cc),
                            start=(cc == 0 or cc == _CHALF),
                            stop=(cc == _CHALF - 1 or cc == NCHUNK - 1),
                        )

                rpre_next = emit_pre(0)
                sc_prev = None
                for t in range(NTILE):
                    g = _TILE_GROUP[t]
                    rpre = rpre_next
                    if t + 1 < NTILE:
                        rpre_next = emit_pre(t + 1)

                    ps_m1 = psB.tile([128, 512], dt, tag="m1", name=f"m1_{t}")
                    for k in range(4):
                        ck = slice(k * HID, (k + 1) * HID)
                        nc.tensor.matmul(ps_m1[:, ck], rpre[:, ck], We2b,
                                         start=True, stop=True)
                    m = wpool.tile([128, 512], bf, tag="m", bufs=6)
                    if be2_zero:
                        nc.scalar.activation(m[:], ps_m1[:], AF.Relu, bias=0.0)
                    else:
                        m1s = wpool.tile([128, 512], dt, tag="m1s", bufs=3)
                        nc.vector.tensor_tensor(m1s[:], ps_m1[:], be2b, ALU.add)
                        nc.scalar.activation(m[:], m1s[:], AF.Relu, bias=0.0)

                    # att[e] = sum_h m[e,h] * Wat[h] (DVE accum along free dim)
                    scr = wpool.tile([128, 512], bf, tag="scr", bufs=4)
                    for k in range(4):
                        ck = slice(k * HID, (k + 1) * HID)
                        cc = t * 4 + k
                        nc.vector.scalar_tensor_tensor(
                            out=scr[:, ck], in0=m[:, ck], scalar=1.0,
                            in1=WatB, op0=ALU.mult, op1=ALU.mult,
                            accum_out=attc[:, cc : cc + 1],
                        )
                    nc.scalar.activation(
                        sigc[:, t * 4 : (t + 1) * 4], attc[:, t * 4 : (t + 1) * 4],
                        AF.Sigmoid, bias=float(bats[l]),
                    )
                    mg = wpool.tile([128, 512], bf, tag="mg", bufs=6)
                    sl4 = slice(t * 4, (t + 1) * 4)
                    nc.vector.tensor_tensor(
                        mg[:].rearrange("p (k h) -> p k h", k=4),
                        m[:].rearrange("p (k h) -> p k h", k=4),
                        sigc[:, sl4].unsqueeze(-1).broadcast_to([HID, 4, HID]),
                        ALU.mult,
                    )
                    # scatter deferred one iteration so its inputs are ready
                    # well before the PE (strict FIFO) reaches it
                    if sc_prev is not None:
                        emit_scatter(*sc_prev)
                    sc_prev = (t, mg)
                emit_scatter(*sc_prev)
                # ---- node MLP:  h += relu([h, agg] @ Wn1 + bn1) @ Wn2 + bn2 ----
                # column-separable: finish i<128 first so the next layer's
                # har matmuls (which consume hTb by column half) start sooner
                nc.vector.tensor_copy(aggb[:], ps_agg[:])
                nc.tensor.matmul(ps_n1[:], Wn1a, hT_cur[:], start=True, stop=False)
                nc.tensor.matmul(ps_n1[:], Wn1b, aggb[:], start=False, stop=True)
                t1 = wpool.tile([HID, N], dt, tag="m1s")
                ps_n2 = psB.tile([HID, N], dt, tag="m1")
                hT_new = lpool.tile([HID, N], dt, tag="hT")
                hTb_new = lpool.tile([HID, N], bf, tag="hTb")
                for hs in (slice(0, HID), slice(HID, N)):
                    nc.scalar.activation(t1[:, hs], ps_n1[:, hs], AF.Relu, bias=bn1)
                    nc.tensor.matmul(ps_n2[:, hs], Wn2, t1[:, hs],
                                     start=True, stop=True)
                    nc.vector.scalar_tensor_tensor(
                        out=hT_new[:, hs], in0=ps_n2[:, hs], scalar=bn2,
                        in1=hT_cur[:, hs], op0=ALU.add, op1=ALU.add,
                    )
                    nc.vector.tensor_copy(hTb_new[:, hs], hT_new[:, hs])
                hT_cur, hTb_cur = hT_new, hTb_new

            # ---- output head ----
            o = NLAYERS * _PL
            W_out = wp[:, o : o + HID]
            W_lin = wp[:, o + HID : o + HID + 1]
            b_out = wp[:, o + HID + 1 : o + HID + 2]
            b_lin = wp[0:1, o + HID + 2 : o + HID + 3]
            ps_o = psA.tile([HID, NS], dt, tag="pre")
            nc.tensor.matmul(ps_o[:], W_out, hT_cur[:, 0:NS], start=True, stop=True)
            ho = wpool.tile([HID, NS], dt, tag="m1s")
            nc.scalar.activation(ho[:], ps_o[:], AF.Relu, bias=b_out)
            ps_y = psB.tile([1, NS], dt, tag="m1")
            nc.tensor.matmul(ps_y[:], W_lin, ho[:], start=True, stop=True)
            y = wpool.tile([1, NS], dt, tag="scr", bufs=4)
            nc.scalar.activation(y[:], ps_y[:], AF.Identity, bias=b_lin)
            nc.sync.dma_start(out_d.ap(), y[:])


def _make_in_maps(inputs, n_layers):
    import ml_dtypes

    bf = ml_dtypes.bfloat16
    h0, d2, adj, mask = _host_prep(inputs)
    wpb, wp, wc3, becb, bats, be2z = _pack_weights(inputs)
    in_maps = []
    for b in range(B):
        Sip, Sjp, d2p3, A = _pack_edges(d2[b], adj[b])
        hTb = np.ascontiguousarray(h0[b].T)
        im = {
            "hT0b": hTb.astype(bf),
            "d2p3": d2p3,
            "wpack": wp, "wc3": wc3,
        }
        for l in range(NLAYERS):
            im[f"wpackb{l}"] = np.ascontiguousarray(
                wpb[:, l * _PLB : (l + 1) * _PLB])
        if not all(bool(z) for z in be2z):
            im["becb"] = becb
        for p in range(_NS_):
            im[f"Sit{p}"] = np.ascontiguousarray(
                Sip[:, _SB[p] * 512 : _SB[p + 1] * 512])
            im[f"Sjt{p}"] = np.ascontiguousarray(
                Sjp[:, _SB[p] * 512 : _SB[p + 1] * 512])
        for p in range(_NA):
            im[f"Ascat{p}"] = np.ascontiguousarray(
                A[:, _AB[p] * HID : _AB[p + 1] * HID])
        in_maps.append(im)
    return in_maps, mask, bats, be2z


def _install_ntff_hook():
    """Recreate the antenv.axon_hooks module the boot expected, register the
    ctypes NTFF hook from trn_agent_boot, so run_bass_kernel_spmd(trace=True)
    can capture hardware profiles under axon."""
    import types

    if "antenv.axon_hooks" not in sys.modules:
        mod = types.ModuleType("antenv.axon_hooks")
        holder = [None]
        mod.set_axon_ntff_profile_hook = lambda h: holder.__setitem__(0, h)
        mod.get_axon_ntff_profile_hook = lambda: holder[0]
        sys.modules["antenv.axon_hooks"] = mod
        import antenv

        antenv.axon_hooks = mod
    m = sys.modules["antenv.axon_hooks"]
    if m.get_axon_ntff_profile_hook() is None:
        sys.path.insert(0, "/root/.axon_site")
        from trn_agent_boot.trn_boot import _ntff_profile_via_ctypes

        m.set_axon_ntff_profile_hook(
            _ntff_profile_via_ctypes("/opt/axon/libaxon_pjrt.so")
        )


_CACHE = {}


def _get_nc(n_layers, bats, be2z):
    key = (n_layers, tuple(np.round(bats, 8)), tuple(be2z))
    if key not in _CACHE:
        import concourse.bass as bass
        import concourse.tile as tile
        from concourse import bacc

        nc = bacc.Bacc(
            "TRN2", target_bir_lowering=False, debug=False, num_devices=B
        )
        _build(nc, tile, bass, n_layers, bats, be2z)
        nc.compile()
        _CACHE[key] = nc
    return _CACHE[key]


def kernel(**inputs):
    inputs = {k: np.asarray(v) for k, v in inputs.items()}
    n_layers = int(os.environ.get("GNN_LAYERS", NLAYERS))
    in_maps, mask, bats, be2z = _make_in_maps(inputs, n_layers)
    nc = _get_nc(n_layers, bats, be2z)

    if os.environ.get("GNN_SIM"):
        from concourse.bass_interp import CoreSim

        outs = []
        for b in range(int(os.environ.get("GNN_SIM_CORES", 1))):
            sim = CoreSim(nc, trace=False)
            for k, v in in_maps[b].items():
                sim.tensor(k)[:] = v
            sim.simulate()
            outs.append(np.array(sim.tensor("out")).reshape(NS, 1))
        while len(outs) < B:
            outs.append(np.zeros((NS, 1), _F32))
        out = np.stack(outs)
    else:
        from concourse.bass_utils import run_bass_kernel_spmd

        if os.environ.get("GNN_TRACE"):
            _install_ntff_hook()
            tmpdir = os.environ.get("GNN_TRACE_DIR") or None
            try:
                res = run_bass_kernel_spmd(
                    nc, in_maps, core_ids=list(range(B)), trace=True, tmpdir=tmpdir
                )
                kernel.last_exec_time_ns = res.exec_time_ns
            except Exception as e:
                print(f"[gnn] traced run failed ({e!r}); retrying untraced")
                res = run_bass_kernel_spmd(nc, in_maps, core_ids=list(range(B)))
        else:
            res = run_bass_kernel_spmd(nc, in_maps, core_ids=list(range(B)))
        kernel.last_results = res
        out = np.stack([r["out"].reshape(NS, 1) for r in res.results])

    return (out * inputs["node_mask"][:, :, None]).astype(_F32)
